# revision 1
# baseline (speedup 1.0000x reference)
"""Squared-L2 distance retrieval kernel (logits[q,p] = ||proto[p]-query[q]||^2)
for Trainium2 via Bass/Tile, data-parallel over 8 NeuronCores.

Math per core (256-query shard, proto replicated):
    logits = ||q||^2 + ||p||^2 - 2 q.p
  - q.p via PE matmuls with the contraction dim (D=1024) on partitions; both
    operands are PE-transposed on device (fp32 has no DMA transpose).
  - ||p||^2 is broadcast into PSUM with a K=1 matmul (lhsT=ones row,
    rhs=p2 row), so the main accumulation chain lands p2 for free.
  - ||q||^2 comes from an ACT square+accumulate pass and is added per-partition
    during the PSUM->SBUF copyback with tensor_scalar.
"""

import numpy as np

B, P, D = 1, 64, 1024
Q = 2048
N_CORES = 8
QSH = Q // N_CORES  # 256 query rows per core
NT = QSH // 128     # m-tiles per core
ND = D // 128       # contraction chunks
QT_GRP = 2          # qT transpose chunks per PSUM tile / copyback

_cache = {}


def _build_nc():
    import concourse.bass as bass
    import concourse.mybir as mybir
    import concourse.tile as tile
    from concourse import bacc
    from concourse.masks import make_identity

    f32 = mybir.dt.float32

    nc = bacc.Bacc("TRN2", target_bir_lowering=False, debug=False)
    query = nc.dram_tensor("query", [QSH, D], f32, kind="ExternalInput").ap()
    proto = nc.dram_tensor("proto", [P, D], f32, kind="ExternalInput").ap()
    logits = nc.dram_tensor("logits", [QSH, P], f32, kind="ExternalOutput").ap()

    with tile.TileContext(nc) as tc:
        with (
            tc.tile_pool(name="const", bufs=1) as const_pool,
            tc.tile_pool(name="work", bufs=1) as work,
            tc.tile_pool(name="acc_ps", bufs=2, space="PSUM") as acc_ps,
            tc.tile_pool(name="qt_ps", bufs=3, space="PSUM") as qt_ps,
            tc.tile_pool(name="pt_ps", bufs=2, space="PSUM") as pt_ps,
        ):
            ident = const_pool.tile([128, 128], f32, tag="ident")
            make_identity(nc, ident[:])
            ones_row = const_pool.tile([1, 128], f32, tag="ones_row")
            nc.vector.memset(ones_row[:], 1.0)

            # --- loads ---
            p_nat = work.tile([P, D], f32, tag="p_nat")
            nc.sync.dma_start(p_nat[:], proto[:, :])
            q_nat = []
            for t in range(NT):
                qn = work.tile([128, D], f32, tag=f"q_nat{t}")
                nc.sync.dma_start(qn[:], query[t * 128:(t + 1) * 128, :])
                q_nat.append(qn)

            # --- squared norms (ACT square + free-dim accumulate) ---
            scratch = work.tile([128, D], f32, tag="scratch")
            q2 = work.tile([128, NT], f32, tag="q2")
            for t in range(NT):
                nc.scalar.activation(
                    scratch[:], q_nat[t][:],
                    mybir.ActivationFunctionType.Square,
                    accum_out=q2[:, t:t + 1],
                )
            p2col = work.tile([P, 1], f32, tag="p2col")
            nc.scalar.activation(
                scratch[:P, :], p_nat[:],
                mybir.ActivationFunctionType.Square,
                accum_out=p2col[:],
            )
            # p2 as a [1, P] row (tiny PE transpose + ACT copyback)
            p2row_ps = pt_ps.tile([1, P], f32, tag="pt")
            nc.tensor.transpose(p2row_ps[:], p2col[:], ident[:P, :P])
            p2row = work.tile([1, P], f32, tag="p2row")
            nc.scalar.copy(p2row[:], p2row_ps[:])

            # --- proto^T, scaled by -2 during copyback ---
            ptneg = work.tile([128, ND, P], f32, tag="ptneg")
            for d in range(ND):
                ps = pt_ps.tile([128, P], f32, tag="pt")
                nc.tensor.transpose(ps[:], p_nat[:, d * 128:(d + 1) * 128], ident[:P, :P])
                nc.scalar.mul(ptneg[:, d, :], ps[:], -2.0)

            # --- per-m-tile: query^T chunks, then the matmul chain ---
            for t in range(NT):
                qt_t = work.tile([128, ND, 128], f32, tag=f"qT{t}")
                for g in range(ND // QT_GRP):
                    ps = qt_ps.tile([128, QT_GRP, 128], f32, tag="qt")
                    for j in range(QT_GRP):
                        d = g * QT_GRP + j
                        nc.tensor.transpose(
                            ps[:, j], q_nat[t][:, d * 128:(d + 1) * 128], ident[:]
                        )
                    nc.vector.tensor_copy(
                        qt_t[:, g * QT_GRP:(g + 1) * QT_GRP, :], ps[:]
                    )

                acc = acc_ps.tile([128, P], f32, tag="acc")
                # psum = broadcast(p2) via K=1 matmul
                nc.tensor.matmul(acc[:], ones_row[:], p2row[:], start=True, stop=False)
                for d in range(ND):
                    nc.tensor.matmul(
                        acc[:], qt_t[:, d, :], ptneg[:, d, :],
                        start=False, stop=(d == ND - 1),
                    )
                out_sb = work.tile([128, P], f32, tag=f"out_sb{t}")
                nc.vector.tensor_scalar_add(out_sb[:], acc[:], q2[:, t:t + 1])
                nc.sync.dma_start(logits[t * 128:(t + 1) * 128, :], out_sb[:])

    nc.compile()
    return nc


def _get_nc():
    if "nc" not in _cache:
        _cache["nc"] = _build_nc()
    return _cache["nc"]


def kernel(**inputs) -> np.ndarray:
    from concourse.bass_utils import run_bass_kernel_spmd

    query = np.ascontiguousarray(
        np.asarray(inputs["query"], dtype=np.float32).reshape(Q, D))
    proto = np.ascontiguousarray(
        np.asarray(inputs["proto"], dtype=np.float32).reshape(P, D))

    nc = _get_nc()
    in_maps = [
        {
            "query": np.ascontiguousarray(query[c * QSH:(c + 1) * QSH]),
            "proto": proto,
        }
        for c in range(N_CORES)
    ]
    res = run_bass_kernel_spmd(nc, in_maps, core_ids=list(range(N_CORES)))
    return np.concatenate([r["logits"] for r in res.results], axis=0)


# revision 33
# speedup vs baseline: 1.2253x; 1.2253x over previous
"""Squared-L2 distance retrieval kernel (logits[q,p] = ||proto[p]-query[q]||^2)
for Trainium2 via Bass/Tile, data-parallel over 8 NeuronCores.

Per core (256-query shard, proto replicated):  logits = ||q||^2+||p||^2-2 q.p
  - q.p via PE matmuls with the contraction dim (D=1024) on partitions;
    the query is PE-transposed on device (fp32 has no DMA transpose).
  - ||p||^2 is broadcast into every PSUM accumulation chain with a K=1 matmul
    (lhsT = ones row, rhs = p2 row).
  - ||q||^2 via square+row-sum (ACT activation+accum / DVE
    tensor_tensor_reduce), added per-partition during the PSUM->SBUF copyback.
Pipelining: all DMAs on the SP HWDGE ring; query arrives in column chunks so
PE transposes stream behind the DMA; dummy PE warmup transposes climb the
clock ramp before real work arrives.

Every construct not validated on hardware is behind a CFG flag so the kernel
can fall back to a conservative variant.
"""

import contextlib

import numpy as np

B, P, D = 1, 64, 1024
Q = 2048
N_CORES = 8
QSH = Q // N_CORES   # 256 query rows per core
NT = QSH // 128      # m-tiles per core
ND = D // 128        # contraction chunks
QT_GRP = 2           # max d-chunks per qT psum group

_cache = {}

CFG = dict(
    n_warmup=6,            # dummy PE transpose pairs (0 = off)
    groups=(4, 4),         # d-chunks per query DMA chunk / qt group
    norm_pieces=(1024,),   # column widths of ||q||^2 partial passes
    copy_mode="tile",      # qt copyback engine: "alt" (g+t)%2 / "tile" per-t
    copy_t0="dve", copy_t1="act",
    norm_t0="act", norm_t1="act",   # ||q||^2 engine per m-tile
    dve_norm_chain=False,  # chain DVE norm pieces via ttr initial-value
    ts_engs=("dve", "dve"),  # final copyback engine per m-tile
    ts_fused=False,        # single tensor_scalar(imm mult, AP add) vs 2 ops
    proto_mode="prepack",  # "prepack": host-transposed proto; "natural"
    ptsq_eng="act",
    use_3d_dma=False,      # combined [128, 2, w] query chunks vs per-tile 2D
    bcast_first=True,      # p2-broadcast matmul first vs last in the chain
    hot_tail=False,        # high-priority endgame ops
    qt_bufs=4,
)

SAFE_CFG = dict(
    n_warmup=0, groups=(2, 2, 2, 2), norm_pieces=(1024,),
    copy_mode="tile", copy_t0="dve", copy_t1="act",
    norm_t0="act", norm_t1="act", dve_norm_chain=False,
    ts_engs=("dve", "dve"), ts_fused=False,
    proto_mode="natural", use_3d_dma=False, bcast_first=True,
    hot_tail=False, qt_bufs=4,
)


def _build_nc(cfg=None):
    import concourse.mybir as mybir
    import concourse.tile as tile
    from concourse import bacc
    from concourse.masks import make_identity

    cfg = dict(CFG, **(cfg or {}))
    f32 = mybir.dt.float32
    Alu = mybir.AluOpType
    Act = mybir.ActivationFunctionType

    groups = cfg["groups"]
    gmax = max(max(groups), QT_GRP)
    assert sum(groups) == ND
    g_start = [sum(groups[:i]) for i in range(len(groups))]
    pieces = cfg["norm_pieces"]
    assert sum(pieces) == D
    p_start = [sum(pieces[:i]) for i in range(len(pieces))]
    prepack = cfg["proto_mode"] == "prepack"

    nc = bacc.Bacc("TRN2", target_bir_lowering=False, debug=False)
    query = nc.dram_tensor("query", [QSH, D], f32, kind="ExternalInput").ap()
    if prepack:
        # proto host-prepacked (weight prepacking) as proto^T in SBUF layout:
        # protoT8[dp, c, p] = proto[p, c*128 + dp]
        proto_in = nc.dram_tensor("protoT8", [128, ND, P], f32,
                                  kind="ExternalInput").ap()
    else:
        proto_in = nc.dram_tensor("proto", [P, D], f32,
                                  kind="ExternalInput").ap()
    logits = nc.dram_tensor("logits", [QSH, P], f32, kind="ExternalOutput").ap()
    query_t = query.rearrange("(t p) d -> p t d", p=128)

    with tile.TileContext(nc) as tc:
        with (
            tc.tile_pool(name="const", bufs=1) as const_pool,
            tc.tile_pool(name="work", bufs=1) as work,
            tc.tile_pool(name="acc_ps", bufs=2, space="PSUM") as acc_ps,
            tc.tile_pool(name="qt_ps", bufs=cfg["qt_bufs"],
                         space="PSUM") as qt_ps,
            tc.tile_pool(name="pt_ps", bufs=1, space="PSUM") as pt_ps,
            tc.tile_pool(name="p2r_ps", bufs=1, space="PSUM") as p2r_ps,
        ):
            ident = const_pool.tile([128, 128], f32, tag="ident")
            make_identity(nc, ident[:])
            ones_row = const_pool.tile([1, 128], f32, tag="ones_row")
            nc.vector.memset(ones_row[:], 1.0)
            if prepack:
                ones_col = const_pool.tile([128, 1], f32, tag="ones_col")
                nc.vector.memset(ones_col[:], 1.0)

            # PE warmup: dummy transposes of the identity while the input DMAs
            # stream, so the PE clock ramp (HAM) finishes before real work.
            for w in range(cfg["n_warmup"]):
                wps = qt_ps.tile([128, gmax, 128], f32, tag="qt",
                                 name=f"w{w}")
                for j in range(QT_GRP):
                    nc.tensor.transpose(wps[:, j], ident[:], ident[:])

            # --- loads: proto first, then query chunks (all SP HWDGE) ---
            if prepack:
                pt = work.tile([128, ND, P], f32, tag="pt")
                nc.sync.dma_start(pt[:], proto_in[:, :, :])
            else:
                p_nat = work.tile([P, D], f32, tag="p_nat")
                nc.sync.dma_start(p_nat[:], proto_in[:, :])
            q_nat = work.tile([128, NT, D], f32, tag="q_nat")
            for gs, gn in zip(g_start, groups):
                sl = slice(gs * 128, (gs + gn) * 128)
                if cfg["use_3d_dma"]:
                    nc.sync.dma_start(q_nat[:, :, sl], query_t[:, :, sl])
                else:
                    for t in range(NT):
                        nc.sync.dma_start(
                            q_nat[:, t, sl],
                            query[t * 128:(t + 1) * 128, sl])

            # --- proto side: p^T tiles + (-p2/2) row ---
            if prepack:
                # ||p||^2 = ones.T @ (p^T)^2: square on ACT/DVE, 8 K=128
                # matmuls accumulate the d-sum into a [1, P] psum row.
                ptsq = work.tile([128, ND, P], f32, tag="ptsq")
                if cfg["ptsq_eng"] == "act":
                    nc.scalar.square(ptsq[:], pt[:])
                else:
                    nc.vector.tensor_tensor(out=ptsq[:], in0=pt[:], in1=pt[:],
                                            op=Alu.mult)
                p2row_ps = p2r_ps.tile([1, P], f32, tag="p2r")
                for c in range(ND):
                    nc.tensor.matmul(p2row_ps[:], ones_col[:], ptsq[:, c, :],
                                     start=(c == 0), stop=(c == ND - 1))
                p2row = work.tile([1, P], f32, tag="p2row")
                nc.scalar.mul(p2row[:], p2row_ps[:], -0.5)
            else:
                # on-device pT: PE transposes into one PSUM bank; ACT copies
                # out; p2 via ACT square+accumulate on the natural layout.
                pt_all = pt_ps.tile([128, ND, P], f32, tag="pt")
                for d in range(ND):
                    nc.tensor.transpose(
                        pt_all[:, d], p_nat[:, d * 128:(d + 1) * 128],
                        ident[:P, :P])
                pt = work.tile([128, ND, P], f32, tag="pt")
                half = ND // 2
                nc.scalar.copy(pt[:, :half, :], pt_all[:, :half, :])
                nc.scalar.copy(pt[:, half:, :], pt_all[:, half:, :])
                p_scr = work.tile([P, D], f32, tag="p_scr")
                p2col = work.tile([P, 1], f32, tag="p2col")
                nc.scalar.activation(p_scr[:], p_nat[:], Act.Square,
                                     accum_out=p2col[:])
                p2row_ps = p2r_ps.tile([1, P], f32, tag="p2r")
                nc.tensor.transpose(p2row_ps[:], p2col[:], ident[:P, :P])
                p2row = work.tile([1, P], f32, tag="p2row")
                nc.scalar.mul(p2row[:], p2row_ps[:], -0.5)

            # --- query^T transposes + copybacks + piecewise ||q||^2 ---
            qt = [work.tile([128, ND, 128], f32, tag=f"qT{t}", name=f"qT{t}")
                  for t in range(NT)]
            q_scr = work.tile([128, max(pieces)], f32, tag="q_scr")
            q_scr1 = work.tile([128, max(pieces)], f32, tag="q_scr1")
            q2p = work.tile([128, NT, max(2, len(pieces))], f32, tag="q2p")
            q2 = work.tile([128, NT], f32, tag="q2")
            norm_done = [0, 0]

            def norm_eng(t):
                return cfg["norm_t0"] if t == 0 else cfg["norm_t1"]

            def emit_norms_ready(cols_avail, t):
                # DVE norm pieces may chain partials via the accum initial
                # value (last piece lands in q2); otherwise per-piece partials
                # are reduced at the end.
                eng = norm_eng(t)
                i = norm_done[t]
                scr = q_scr if t == 0 else q_scr1
                while i < len(pieces) and p_start[i] + pieces[i] <= cols_avail:
                    sl = slice(p_start[i], p_start[i] + pieces[i])
                    src = q_nat[:, t, sl]
                    last = i == len(pieces) - 1
                    chain = eng == "dve" and cfg["dve_norm_chain"]
                    dst = (q2[:, t:t + 1]
                           if (last and (chain or len(pieces) == 1))
                           else q2p[:, t, i:i + 1])
                    if eng == "act":
                        nc.scalar.activation(scr[:, :src.shape[-1]], src,
                                             Act.Square, accum_out=dst)
                    else:
                        init = (q2p[:, t, i - 1:i] if (chain and i > 0)
                                else 0.0)
                        nc.vector.tensor_tensor_reduce(
                            out=scr[:, :src.shape[-1]], in0=src, in1=src,
                            scale=1.0, scalar=init,
                            op0=Alu.mult, op1=Alu.add, accum_out=dst)
                    i += 1
                norm_done[t] = i

            for gi, (gs, gn) in enumerate(zip(g_start, groups)):
                hot = (tc.high_priority()
                       if cfg["hot_tail"] and gi >= len(groups) - 2
                       else contextlib.nullcontext())
                with hot:
                    for t in range(NT):
                        ps = qt_ps.tile([128, gmax, 128], f32, tag="qt")
                        for j in range(gn):
                            d = gs + j
                            nc.tensor.transpose(
                                ps[:, j], q_nat[:, t, d * 128:(d + 1) * 128],
                                ident[:])
                        dst = qt[t][:, gs:gs + gn, :]
                        if cfg["copy_mode"] == "alt":
                            eng = "dve" if (gi + t) % 2 == 0 else "act"
                        else:
                            eng = cfg["copy_t0"] if t == 0 else cfg["copy_t1"]
                        if eng == "dve":
                            nc.vector.tensor_copy(dst, ps[:, :gn])
                        else:
                            nc.scalar.copy(dst, ps[:, :gn])
                        emit_norms_ready((gs + gn) * 128, t)

            for t in range(NT):
                chain = norm_eng(t) == "dve" and cfg["dve_norm_chain"]
                if not chain and len(pieces) > 1:
                    nc.vector.reduce_sum(q2[:, t:t + 1],
                                         q2p[:, t, :len(pieces)],
                                         axis=mybir.AxisListType.X)

            # --- matmul chains + final copyback + output DMAs ---
            out_sb = work.tile([128, NT, P], f32, tag="out_sb")
            for t in range(NT):
                acc = acc_ps.tile([128, P], f32, tag="acc")
                if cfg["bcast_first"]:
                    nc.tensor.matmul(acc[:], ones_row[:], p2row[:],
                                     start=True, stop=False)
                for d in range(ND):
                    nc.tensor.matmul(
                        acc[:], qt[t][:, d, :], pt[:, d, :],
                        start=(d == 0 and not cfg["bcast_first"]),
                        stop=(d == ND - 1 and cfg["bcast_first"]))
                if not cfg["bcast_first"]:
                    nc.tensor.matmul(acc[:], ones_row[:], p2row[:],
                                     start=False, stop=True)
                # out = -2*(qp - p2/2) + q2 = q2 + p2 - 2 qp
                hp = (tc.high_priority() if cfg["hot_tail"]
                      else contextlib.nullcontext())
                with hp:
                    if cfg["ts_fused"]:
                        if cfg["ts_engs"][t] == "dve":
                            nc.vector.tensor_scalar(
                                out_sb[:, t, :], acc[:], -2.0, q2[:, t:t + 1],
                                op0=Alu.mult, op1=Alu.add)
                        else:
                            nc.scalar.activation(
                                out_sb[:, t, :], acc[:], Act.Identity,
                                bias=q2[:, t:t + 1], scale=-2.0)
                    else:
                        nc.scalar.mul(out_sb[:, t, :], acc[:], -2.0)
                        nc.vector.tensor_scalar_add(
                            out_sb[:, t, :], out_sb[:, t, :], q2[:, t:t + 1])
                    nc.sync.dma_start(
                        logits[t * 128:(t + 1) * 128, :], out_sb[:, t, :])

    nc.compile()
    return nc


def _core_inputs(query, proto, cfg=None):
    cfg = dict(CFG, **(cfg or {}))
    if cfg["proto_mode"] == "prepack":
        pk = np.ascontiguousarray(proto.reshape(P, ND, 128).transpose(2, 1, 0))
        pmap = {"protoT8": pk}
    else:
        pmap = {"proto": np.ascontiguousarray(proto)}
    return [
        dict(query=np.ascontiguousarray(query[c * QSH:(c + 1) * QSH]), **pmap)
        for c in range(N_CORES)
    ]


def _get_nc():
    if "nc" not in _cache:
        _cache["nc"] = _build_nc()
    return _cache["nc"]


def kernel(**inputs) -> np.ndarray:
    from concourse.bass_utils import run_bass_kernel_spmd

    query = np.ascontiguousarray(
        np.asarray(inputs["query"], dtype=np.float32).reshape(Q, D))
    proto = np.asarray(inputs["proto"], dtype=np.float32).reshape(P, D)

    nc = _get_nc()
    in_maps = _core_inputs(query, proto)
    res = run_bass_kernel_spmd(nc, in_maps, core_ids=list(range(N_CORES)))
    return np.concatenate([r["logits"] for r in res.results], axis=0)
